# revision 27
# baseline (speedup 1.0000x reference)
"""BehaviorAwareGCNLayer on 8 Trainium2 NeuronCores.

Math (reference):
    hx  = x @ W
    out[r] = (1/deg[r]) * sum_{e: row[e]=r} sim_w[e]*sigmoid(rep[row]+rep[col])*ns[col] * hx[col]
    out += sigmoid(rep) * (x @ W_self);  leaky_relu(out, 0.01)

Device strategy (destination sharding, no collectives):
  - By linearity, W is applied AFTER aggregation: agg[r] = sum coef_e * x[col_e],
    out[r] = (agg[r]/deg[r]) @ W + sigmoid(rep_r)*(x_r @ W_self).
  - Host does LAYOUT only (grouping/padding/fancy-index copies and structural
    edge counts); all value math (sigmoid, products, sums, matmuls) happens on
    device.
  - Core c owns destination rows [c*12500, (c+1)*12500). Edges are grouped
    into runs by (core, 128-row destination block, 32768-row source
    col-range), padded to a 32-edge quantum with run capacities uniform
    across cores -> single SPMD program.
  - Blocks are striped into G groups; chunk order is (group, range)-major so
    each dma_gather instruction reads one 32768-row window of x with
    all-valid int16 indices, while early block groups finish (and finalize)
    before the gather stream ends. Gathers rotate across the 4 SWDGE queues
    so descriptor generation pipelines across all four Q7 core pairs.
  - Runs are split at chunk boundaries into SEGMENTS (one matmul each). Per
    segment a 128-wide bf16 one-hot S[e, j] = (row_off[e] == j) is built on
    DVE (slots outside the segment's run carry a dummy row offset, so pad
    slots and other-run slots contribute zero). Messages are bf16:
    xs[e, :] = coef_e * x[col_e] (single tensor_tensor, f32 in / bf16 out).
  - Each destination block accumulates directly in PSUM across all its
    segments in the group sweep: psum_b[j, :] += S^T @ xs. deg comes from a
    host-side structural bincount, so no ones-column is needed.
  - Per block finalize: recip(deg+1e-6), cat = [agg*recip | sigmoid(rep)*x],
    one PE transpose + one matmul with [W; W_self] applies both weight
    matrices, leaky_relu, DMA out.
"""
import sys

if "/opt/trn_rl_repo" not in sys.path:
    sys.path.insert(0, "/opt/trn_rl_repo")

import numpy as np

P = 128
D = 64
N_NODES = 100000
N_CORES = 8
N_LOC = N_NODES // N_CORES            # 12500 destination rows per core
N_BLK = (N_LOC + P - 1) // P          # 98 blocks per core
LAST_VALID = N_LOC - (N_BLK - 1) * P  # 84 valid rows in last block
RANGE = 32768                         # int16-addressable source window
N_RANGES = (N_NODES + RANGE - 1) // RANGE  # 4
BATCH = 32                            # chunks per compute batch
GCH = 8                               # chunks per dma_gather
QUANT = 1                             # run padding quantum (none needed)
N_GRP = 4                             # block stripes (finalize overlap)
DUMMY_OFF = 1000.0                    # one-hot-killing row offset for pad slots


def _layout(cap32):
    """Derive the uniform slot/segment layout from per-(block, range)
    capacities. cap32[b][r]: run capacity in edges (multiple of QUANT)."""
    n_blk = len(cap32)
    n_ranges = len(cap32[0])
    grp_of = [min(b * N_GRP // n_blk, N_GRP - 1) for b in range(n_blk)]
    groups = [[b for b in range(n_blk) if grp_of[b] == g] for g in range(N_GRP)]

    run_start = [[0] * n_ranges for _ in range(n_blk)]
    spans = []   # (range, start_slot, end_slot), 128-aligned
    runs = []    # (start_slot, end_slot, block)
    pos = 0
    for g in range(N_GRP):
        for r in range(n_ranges):
            span_start = pos
            for b in groups[g]:
                run_start[b][r] = pos
                cap = int(cap32[b][r])
                if cap:
                    runs.append((pos, pos + cap, b))
                pos += cap
            pos = -(-pos // P) * P  # pad span to chunk boundary
            if pos > span_start:
                spans.append((r, span_start, pos))
    total_slots = pos
    n_chunks = total_slots // P

    # segments: run pieces split at chunk boundaries, in chunk order.
    # A run's segments are consecutive (runs occupy consecutive slot ranges).
    seg_raw = []  # (chunk, lo_in_chunk, block, run_id, run_start, run_stop)
    for ri, (s, e, b) in enumerate(runs):
        cs, ce = s // P, (e - 1) // P
        for ci in range(cs, ce + 1):
            seg_raw.append((ci, max(s, ci * P) - ci * P, b, ri,
                            ci == cs, ci == ce))
    seg_raw.sort()
    blk_last_seg = {}
    for si, (ci, lo, b, ri, rs, re) in enumerate(seg_raw):
        blk_last_seg[b] = si
    segments = []  # (chunk, block, run_start, run_stop, finalize_after)
    for si, (ci, lo, b, ri, rs, re) in enumerate(seg_raw):
        segments.append((ci, b, rs, re, blk_last_seg[b] == si))

    # (block, chunk) -> segment id (for host metadata fill)
    seg_of = {}
    for si, (ci, lo, b, ri, rs, re) in enumerate(seg_raw):
        seg_of[(b, ci)] = si
    return dict(total_slots=total_slots, run_start=run_start, spans=spans,
                segments=segments, seg_of=seg_of, n_chunks=n_chunks,
                n_segs=len(segments))


def _build_program(n_tab, n_blk, cap32, last_valid):
    """Emit + compile the single-core SPMD program."""
    import concourse.bacc as bacc
    import concourse.mybir as mybir
    import concourse.tile as tile
    from concourse.masks import make_identity

    f32 = mybir.dt.float32
    bf16 = mybir.dt.bfloat16
    i16 = mybir.dt.int16
    i32 = mybir.dt.int32

    lay = _layout(cap32)
    C = lay["n_chunks"]
    S = lay["n_segs"]
    segments = lay["segments"]

    nc = bacc.Bacc("TRN2", target_bir_lowering=False, debug=False,
                   num_swdge_queues=4)

    x_d = nc.dram_tensor("x", [n_tab, D], f32, kind="ExternalInput")
    idx_d = nc.dram_tensor("idx16", [P, C * 8], i16, kind="ExternalInput")
    rowoff_d = nc.dram_tensor("rowoff_t", [P, S], bf16, kind="ExternalInput")
    sw_d = nc.dram_tensor("sw_t", [P, C], f32, kind="ExternalInput")
    reprow_d = nc.dram_tensor("reprow_t", [P, C], f32, kind="ExternalInput")
    repc_d = nc.dram_tensor("repc_t", [P, C], f32, kind="ExternalInput")
    nsc_d = nc.dram_tensor("nsc_t", [P, C], f32, kind="ExternalInput")
    repsh_d = nc.dram_tensor("rep_sh", [P, n_blk], f32, kind="ExternalInput")
    degsh_d = nc.dram_tensor("deg_sh", [P, n_blk], f32, kind="ExternalInput")
    xself_d = nc.dram_tensor("x_self", [n_blk * P, D], f32, kind="ExternalInput")
    wcat_d = nc.dram_tensor("w_cat", [2 * D, D], f32, kind="ExternalInput")
    out_d = nc.dram_tensor("out", [n_blk * P, D], f32, kind="ExternalOutput")

    AL = mybir.AluOpType
    ACT = mybir.ActivationFunctionType

    # batches: within gather spans, never crossing a range boundary;
    # attach the segment id range of each batch
    batches = []  # (c0, nb, range, s0, s1)
    seg_ci = [s[0] for s in segments]
    for (r, s0s, s1s) in lay["spans"]:
        cs, ce = s0s // P, s1s // P
        for c0 in range(cs, ce, BATCH):
            nb = min(BATCH, ce - c0)
            import bisect
            sa = bisect.bisect_left(seg_ci, c0)
            sb = bisect.bisect_left(seg_ci, c0 + nb)
            batches.append((c0, nb, r, sa, sb))
    seg_max = max(sb - sa for (_, _, _, sa, sb) in batches)

    with tile.TileContext(nc) as tc:
        with (
            tc.tile_pool(name="meta", bufs=1) as meta,
            tc.tile_pool(name="idxp", bufs=4) as idxp,
            tc.tile_pool(name="gather", bufs=6) as gpool,
            tc.tile_pool(name="work", bufs=4) as wpool,
            tc.tile_pool(name="onehot", bufs=4) as opool,
            tc.tile_pool(name="const", bufs=1) as cpool,
            tc.tile_pool(name="fin", bufs=3) as fpool,
            tc.tile_pool(name="psum", bufs=4, space="PSUM") as psum,
            tc.tile_pool(name="psumT", bufs=2, space="PSUM") as psumT,
        ):
            rowoff_s = meta.tile([P, S], bf16)
            sw_s = meta.tile([P, C], f32)
            reprow_s = meta.tile([P, C], f32)
            repc_s = meta.tile([P, C], f32)
            nsc_s = meta.tile([P, C], f32)
            repsh_s = meta.tile([P, n_blk], f32)
            degsh_s = meta.tile([P, n_blk], f32)
            acc_all = meta.tile([P, n_blk, D], f32)
            wcat_s = cpool.tile([2 * D, D], f32)
            ident = cpool.tile([P, P], f32)
            iota_i = cpool.tile([P, P], i32)
            iota_f = cpool.tile([P, P], bf16)
            nc.sync.dma_start(out=rowoff_s[:], in_=rowoff_d[:])
            nc.sync.dma_start(out=sw_s[:], in_=sw_d[:])
            nc.sync.dma_start(out=reprow_s[:], in_=reprow_d[:])
            nc.sync.dma_start(out=repc_s[:], in_=repc_d[:])
            nc.sync.dma_start(out=nsc_s[:], in_=nsc_d[:])
            nc.sync.dma_start(out=repsh_s[:], in_=repsh_d[:])
            nc.sync.dma_start(out=degsh_s[:], in_=degsh_d[:])
            nc.sync.dma_start(out=wcat_s[:], in_=wcat_d[:])
            nc.vector.memset(acc_all[:].rearrange("p b d -> p (b d)"), 0.0)
            make_identity(nc, ident[:])
            nc.gpsimd.iota(iota_i[:], pattern=[[1, P]], base=0,
                           channel_multiplier=0)
            nc.vector.tensor_copy(out=iota_f[:], in_=iota_i[:])

            run_psum = {}  # block -> live psum tile for its current run

            def finalize_block(blk):
                valid = P if blk < n_blk - 1 else last_valid
                recip = fpool.tile([P, 1], f32, tag="recip")
                nc.any.tensor_scalar_add(out=recip[:],
                                         in0=degsh_s[:, blk:blk + 1],
                                         scalar1=1e-6)
                nc.vector.reciprocal(out=recip[:], in_=recip[:])
                xb = fpool.tile([P, D], f32, tag="xb")
                nc.scalar.dma_start(out=xb[:], in_=xself_d[blk * P:(blk + 1) * P, :])
                srep = fpool.tile([P, 1], f32, tag="srep")
                nc.scalar.activation(srep[:], repsh_s[:, blk:blk + 1], ACT.Sigmoid)
                cat = fpool.tile([P, 2 * D], f32, tag="cat")
                nc.scalar.mul(out=cat[:, 0:D], in_=acc_all[:, blk, :],
                              mul=recip[:])
                nc.scalar.mul(out=cat[:, D:2 * D], in_=xb[:], mul=srep[:])
                catT_ps = psumT.tile([P, P], f32, tag="catT")
                nc.tensor.transpose(out=catT_ps[:], in_=cat[:], identity=ident[:])
                catT = fpool.tile([P, P], f32, tag="catT_s")
                nc.vector.tensor_copy(out=catT[:], in_=catT_ps[:])
                out_ps = psumT.tile([P, D], f32, tag="out_ps")
                nc.tensor.matmul(out=out_ps[:], lhsT=catT[:], rhs=wcat_s[:],
                                 start=True, stop=True)
                outb = fpool.tile([P, D], f32, tag="outb")
                lk = fpool.tile([P, D], f32, tag="lk")
                nc.scalar.mul(out=lk[:], in_=out_ps[:], mul=0.01)
                nc.any.tensor_tensor(out=outb[:], in0=out_ps[:], in1=lk[:],
                                     op=AL.max)
                nc.scalar.dma_start(out=out_d[blk * P:blk * P + valid, :],
                                     in_=outb[:valid, :])

            gq = 0  # rotating SWDGE queue
            fin_order = []  # finalize everything after the gather stream
            for (c0, nb, r, sa, sb) in batches:
                idx_t = idxp.tile([P, BATCH * 8], i16, tag="idx")
                nc.sync.dma_start(out=idx_t[:, :nb * 8],
                                  in_=idx_d[:, c0 * 8:(c0 + nb) * 8])
                xg = gpool.tile([P, BATCH, D], f32, tag="xg")
                for s in range(0, nb, GCH):
                    ns = min(GCH, nb - s)
                    nc.gpsimd.dma_gather(
                        out_ap=xg[:, s:s + ns, :], in_ap=x_d[r * RANGE:, :],
                        idxs_ap=idx_t[:, s * 8:(s + ns) * 8],
                        num_idxs=ns * P, num_idxs_reg=ns * P, elem_size=D,
                        queue_num=gq % 4)
                    gq += 1

                # coef = sw * sigmoid(rep_row + rep_col) * ns_col   [P, nb]
                coef = wpool.tile([P, BATCH], f32, tag="coef")
                nc.any.tensor_tensor(out=coef[:, :nb],
                                     in0=reprow_s[:, c0:c0 + nb],
                                     in1=repc_s[:, c0:c0 + nb], op=AL.add)
                nc.scalar.activation(coef[:, :nb], coef[:, :nb], ACT.Sigmoid)
                nc.any.tensor_tensor(out=coef[:, :nb], in0=coef[:, :nb],
                                     in1=sw_s[:, c0:c0 + nb], op=AL.mult)
                nc.any.tensor_tensor(out=coef[:, :nb], in0=coef[:, :nb],
                                     in1=nsc_s[:, c0:c0 + nb], op=AL.mult)

                # bf16 messages: xs = coef * x_gathered (cast fused)
                xs2 = wpool.tile([P, BATCH, D], bf16, tag="xs2")
                nc.vector.tensor_tensor(
                    out=xs2[:, :nb, :], in0=xg[:, :nb, :],
                    in1=coef[:, :nb].rearrange("p (b o) -> p b o", o=1)
                        .to_broadcast([P, nb, D]),
                    op=AL.mult)

                # 128-wide one-hots, one per segment in this batch
                nseg = sb - sa
                oh = opool.tile([P, seg_max, P], bf16, tag="oh")
                nc.vector.tensor_tensor(
                    out=oh[:, :nseg, :],
                    in0=rowoff_s[:, sa:sb]
                        .rearrange("p (b o) -> p b o", o=1)
                        .to_broadcast([P, nseg, P]),
                    in1=iota_f[:].rearrange("p (b n) -> p b n", b=1)
                        .to_broadcast([P, nseg, P]),
                    op=AL.is_equal)

                for si in range(sa, sb):
                    ci, blk, rstart, rstop, fin = segments[si]
                    if rstart:
                        run_psum[blk] = psum.tile([P, D], f32, tag="agg",
                                                  name="agg_ps")
                    nc.tensor.matmul(
                        out=run_psum[blk][:],
                        lhsT=oh[:, si - sa, :],
                        rhs=xs2[:, ci - c0, :],
                        start=rstart, stop=rstop)
                    if rstop:
                        nc.any.tensor_tensor(
                            out=acc_all[:, blk, :], in0=acc_all[:, blk, :],
                            in1=run_psum.pop(blk)[:], op=AL.add)
                    if fin:
                        fin_order.append(blk)
            for blk in fin_order:
                finalize_block(blk)

    # blocks with no edges at all (never happens statistically)
    lay_blocks = {b for (_, b, _, _, _) in segments}
    assert len(lay_blocks) == n_blk, "empty block needs memset finalize"

    nc.compile()
    return nc


def _preprocess(x, edge_index, sim_weight, rep, node_signal):
    """Host-side layout: group edges into (core, dest block, col range) runs,
    pad to uniform 32-edge-quantum capacities, produce per-core arrays."""
    import ml_dtypes

    row = np.ascontiguousarray(edge_index[0]).astype(np.int64)
    col = np.ascontiguousarray(edge_index[1]).astype(np.int64)
    sw = np.ascontiguousarray(sim_weight).astype(np.float32)
    rep_f = np.ascontiguousarray(rep).astype(np.float32)
    ns_f = np.ascontiguousarray(node_signal).astype(np.float32)
    E = row.shape[0]
    rng_e = col // RANGE

    # Balanced row->(core, block) assignment: group rows by their per-range
    # edge-count signature and deal them round-robin over the 784 core-block
    # bins, so per-(block, range) counts are nearly equal across cores and the
    # max-over-cores run capacities carry almost no padding. Pure layout
    # (integer counting + permutation).
    n_bins = N_CORES * N_BLK
    hist = np.zeros((N_NODES, N_RANGES), dtype=np.int64)
    np.add.at(hist, (row, rng_e), 1)
    sig = ((hist[:, 0] * 64 + hist[:, 1]) * 64 + hist[:, 2]) * 64 + hist[:, 3]
    order_r = np.argsort(sig, kind="stable")
    bin_of = np.empty(N_NODES, dtype=np.int64)
    bin_of[order_r] = np.arange(N_NODES) % n_bins
    cap_bin = np.full(n_bins, P, dtype=np.int64)
    cap_bin[n_bins - N_CORES:] = LAST_VALID   # last block of each core
    cnts_b = np.bincount(bin_of, minlength=n_bins)
    spill = []
    free_b = cap_bin - cnts_b
    for b in np.where(free_b < 0)[0]:
        rows_b = np.where(bin_of == b)[0]
        spill.extend(rows_b[: (-free_b[b])])
    free_b = np.maximum(free_b, 0)
    fill_bins = np.repeat(np.arange(n_bins), free_b)
    if spill:
        bin_of[np.array(spill, dtype=np.int64)] = fill_bins[: len(spill)]
    # rank within bin -> local row
    order_b = np.argsort(bin_of, kind="stable")
    starts = np.zeros(n_bins + 1, dtype=np.int64)
    np.cumsum(np.bincount(bin_of, minlength=n_bins), out=starts[1:])
    rank_r = np.empty(N_NODES, dtype=np.int64)
    rank_r[order_b] = np.arange(N_NODES) - starts[bin_of[order_b]]
    corem = bin_of % N_CORES
    blkm = bin_of // N_CORES
    lrowm = blkm * P + rank_r
    orig_of = np.zeros((N_CORES, N_BLK * P), dtype=np.int64)
    orig_of[corem, lrowm] = np.arange(N_NODES)

    core = corem[row]
    blk = blkm[row]
    off = (lrowm[row] & 127).astype(np.float32)

    counts = np.zeros((N_CORES, N_BLK, N_RANGES), dtype=np.int64)
    np.add.at(counts, (core, blk, rng_e), 1)
    cap32 = (-(-counts.max(axis=0) // QUANT) * QUANT).astype(np.int64)

    lay = _layout(cap32)
    C = lay["n_chunks"]
    S = lay["n_segs"]
    total = lay["total_slots"]
    run_start = np.array(lay["run_start"], dtype=np.int64)  # [N_BLK, N_RANGES]

    # segment id lookup per (block, chunk)
    seg_map = np.full((N_BLK, C), -1, dtype=np.int64)
    for (b, ci), si in lay["seg_of"].items():
        seg_map[b, ci] = si

    key = (core * N_BLK + blk) * N_RANGES + rng_e
    n_groups = N_CORES * N_BLK * N_RANGES
    order = np.argsort(key, kind="stable")
    gcounts = np.bincount(key, minlength=n_groups)
    group_start = np.zeros(n_groups + 1, dtype=np.int64)
    np.cumsum(gcounts, out=group_start[1:])
    rank = np.arange(E, dtype=np.int64) - group_start[key[order]]
    ko = key[order]
    core_o = ko // (N_BLK * N_RANGES)
    blk_o = (ko // N_RANGES) % N_BLK
    rng_o = ko % N_RANGES
    lslot = run_start[blk_o, rng_o] + rank          # slot within core layout
    slot = core_o * total + lslot                   # chunk-layout position
    sid = seg_map[blk_o, lslot >> 7]
    segslot = (core_o * S + sid) * P + (lslot & 127)

    tot = N_CORES * total
    idx_flat = np.zeros(tot, dtype=np.int16)
    sw_p = np.zeros(tot, dtype=np.float32)
    reprow_p = np.zeros(tot, dtype=np.float32)
    repc_p = np.zeros(tot, dtype=np.float32)
    nsc_p = np.zeros(tot, dtype=np.float32)
    idx_flat[slot] = (col[order] - rng_o * RANGE).astype(np.int16)
    sw_p[slot] = sw[order]
    reprow_p[slot] = rep_f[row[order]]
    repc_p[slot] = rep_f[col[order]]
    nsc_p[slot] = ns_f[col[order]]

    rowoff_p = np.full(N_CORES * S * P, DUMMY_OFF, dtype=np.float32)
    rowoff_p[segslot] = off[order]

    def per_core(a):
        return np.ascontiguousarray(a.reshape(N_CORES, C, P).transpose(0, 2, 1))

    sw_t = per_core(sw_p)
    reprow_t = per_core(reprow_p)
    repc_t = per_core(repc_p)
    nsc_t = per_core(nsc_p)
    rowoff_t = np.ascontiguousarray(
        rowoff_p.reshape(N_CORES, S, P).transpose(0, 2, 1)
    ).astype(ml_dtypes.bfloat16)

    idx_w = idx_flat.reshape(N_CORES, C * 8, 16).transpose(0, 2, 1)
    idx16 = np.ascontiguousarray(np.tile(idx_w, (1, 8, 1)))

    rep_pad = np.zeros((N_CORES, N_BLK * P), dtype=np.float32)
    deg_pad = np.zeros((N_CORES, N_BLK * P), dtype=np.float32)
    deg_all = np.bincount(row, minlength=N_NODES).astype(np.float32)
    rep_pad[corem, lrowm] = rep_f
    deg_pad[corem, lrowm] = deg_all
    rep_sh = np.ascontiguousarray(
        rep_pad.reshape(N_CORES, N_BLK, P).transpose(0, 2, 1))
    deg_sh = np.ascontiguousarray(
        deg_pad.reshape(N_CORES, N_BLK, P).transpose(0, 2, 1))

    x_f = np.ascontiguousarray(x).astype(np.float32)
    x_self = np.zeros((N_CORES, N_BLK * P, D), dtype=np.float32)
    x_self[corem, lrowm] = x_f

    return (cap32, x_f, idx16, rowoff_t, sw_t, reprow_t, repc_t, nsc_t,
            rep_sh, deg_sh, x_self, orig_of)


_compiled = {}


def _get_program(cap32):
    key = (N_NODES, N_BLK, LAST_VALID, tuple(map(tuple, cap32.tolist())))
    if key not in _compiled:
        _compiled[key] = _build_program(N_NODES, N_BLK, cap32, LAST_VALID)
    return _compiled[key]


def run(x, edge_index, sim_weight, rep, node_signal, W, W_self, trace=False):
    from concourse.bass_utils import run_bass_kernel_spmd

    (cap32, x_f, idx16, rowoff_t, sw_t, reprow_t, repc_t, nsc_t, rep_sh,
     deg_sh, x_self, orig_of) = _preprocess(x, edge_index, sim_weight, rep,
                                            node_signal)
    w_cat = np.ascontiguousarray(
        np.concatenate([np.asarray(W, dtype=np.float32),
                        np.asarray(W_self, dtype=np.float32)], axis=0))
    nc = _get_program(cap32)
    in_maps = []
    for c in range(N_CORES):
        in_maps.append({
            "x": x_f,
            "idx16": idx16[c],
            "rowoff_t": rowoff_t[c],
            "sw_t": sw_t[c],
            "reprow_t": reprow_t[c],
            "repc_t": repc_t[c],
            "nsc_t": nsc_t[c],
            "rep_sh": rep_sh[c],
            "deg_sh": deg_sh[c],
            "x_self": x_self[c],
            "w_cat": w_cat,
        })
    res = run_bass_kernel_spmd(nc, in_maps, core_ids=list(range(N_CORES)),
                               trace=trace)
    out = np.empty((N_NODES, D), dtype=np.float32)
    for c in range(N_CORES):
        out[orig_of[c, :N_LOC]] = res.results[c]["out"][:N_LOC]
    return out, res


def kernel(x, edge_index, sim_weight, rep, node_signal, W, W_self):
    out, _ = run(x, edge_index, sim_weight, rep, node_signal, W, W_self)
    return out


# revision 29
# speedup vs baseline: 1.2266x; 1.2266x over previous
"""BehaviorAwareGCNLayer on 8 Trainium2 NeuronCores.

Math (reference):
    hx  = x @ W
    out[r] = (1/deg[r]) * sum_{e: row[e]=r} sim_w[e]*sigmoid(rep[row]+rep[col])*ns[col] * hx[col]
    out += sigmoid(rep) * (x @ W_self);  leaky_relu(out, 0.01)

Device strategy (destination sharding, no collectives):
  - By linearity, W is applied AFTER aggregation: agg[r] = sum coef_e * x[col_e],
    out[r] = (agg[r]/deg[r]) @ W + sigmoid(rep_r)*(x_r @ W_self).
  - Host does LAYOUT only (grouping/padding/fancy-index copies and structural
    edge counts); all value math (sigmoid, products, sums, matmuls) happens on
    device.
  - Core c owns destination rows [c*12500, (c+1)*12500). Edges are grouped
    into runs by (core, 128-row destination block, 32768-row source
    col-range), padded to a 32-edge quantum with run capacities uniform
    across cores -> single SPMD program.
  - Blocks are striped into G groups; chunk order is (group, range)-major so
    each dma_gather instruction reads one 32768-row window of x with
    all-valid int16 indices, while early block groups finish (and finalize)
    before the gather stream ends. Gathers rotate across the 4 SWDGE queues
    so descriptor generation pipelines across all four Q7 core pairs.
  - Runs are split at chunk boundaries into SEGMENTS (one matmul each). Per
    segment a 128-wide bf16 one-hot S[e, j] = (row_off[e] == j) is built on
    DVE (slots outside the segment's run carry a dummy row offset, so pad
    slots and other-run slots contribute zero). Messages are bf16:
    xs[e, :] = coef_e * x[col_e] (single tensor_tensor, f32 in / bf16 out).
  - Each destination block accumulates directly in PSUM across all its
    segments in the group sweep: psum_b[j, :] += S^T @ xs. deg comes from a
    host-side structural bincount, so no ones-column is needed.
  - Per block finalize: recip(deg+1e-6), cat = [agg*recip | sigmoid(rep)*x],
    one PE transpose + one matmul with [W; W_self] applies both weight
    matrices, leaky_relu, DMA out.
"""
import sys

if "/opt/trn_rl_repo" not in sys.path:
    sys.path.insert(0, "/opt/trn_rl_repo")

import numpy as np

P = 128
D = 64
N_NODES = 100000
N_CORES = 8
N_LOC = N_NODES // N_CORES            # 12500 destination rows per core
N_BLK = (N_LOC + P - 1) // P          # 98 blocks per core
LAST_VALID = N_LOC - (N_BLK - 1) * P  # 84 valid rows in last block
RANGE = 32768                         # int16-addressable source window
N_RANGES = (N_NODES + RANGE - 1) // RANGE  # 4
BATCH = 32                            # chunks per compute batch
GCH = 8                               # chunks per dma_gather
QUANT = 1                             # run padding quantum (none needed)
N_GRP = 4                             # block stripes (finalize overlap)
DUMMY_OFF = 1000.0                    # one-hot-killing row offset for pad slots


def _layout(cap32):
    """Derive the uniform slot/segment layout from per-(block, range)
    capacities. cap32[b][r]: run capacity in edges (multiple of QUANT)."""
    n_blk = len(cap32)
    n_ranges = len(cap32[0])
    grp_of = [min(b * N_GRP // n_blk, N_GRP - 1) for b in range(n_blk)]
    groups = [[b for b in range(n_blk) if grp_of[b] == g] for g in range(N_GRP)]

    run_start = [[0] * n_ranges for _ in range(n_blk)]
    spans = []   # (range, start_slot, end_slot), 128-aligned
    runs = []    # (start_slot, end_slot, block)
    pos = 0
    for g in range(N_GRP):
        for r in range(n_ranges):
            span_start = pos
            for b in groups[g]:
                run_start[b][r] = pos
                cap = int(cap32[b][r])
                if cap:
                    runs.append((pos, pos + cap, b))
                pos += cap
            pos = -(-pos // P) * P  # pad span to chunk boundary
            if pos > span_start:
                spans.append((r, span_start, pos))
    total_slots = pos
    n_chunks = total_slots // P

    # segments: run pieces split at chunk boundaries, in chunk order.
    # A run's segments are consecutive (runs occupy consecutive slot ranges).
    seg_raw = []  # (chunk, lo_in_chunk, block, run_id, run_start, run_stop)
    for ri, (s, e, b) in enumerate(runs):
        cs, ce = s // P, (e - 1) // P
        for ci in range(cs, ce + 1):
            seg_raw.append((ci, max(s, ci * P) - ci * P, b, ri,
                            ci == cs, ci == ce))
    seg_raw.sort()
    blk_last_seg = {}
    for si, (ci, lo, b, ri, rs, re) in enumerate(seg_raw):
        blk_last_seg[b] = si
    segments = []  # (chunk, block, run_start, run_stop, finalize_after)
    for si, (ci, lo, b, ri, rs, re) in enumerate(seg_raw):
        segments.append((ci, b, rs, re, blk_last_seg[b] == si))

    # (block, chunk) -> segment id (for host metadata fill)
    seg_of = {}
    for si, (ci, lo, b, ri, rs, re) in enumerate(seg_raw):
        seg_of[(b, ci)] = si
    return dict(total_slots=total_slots, run_start=run_start, spans=spans,
                segments=segments, seg_of=seg_of, n_chunks=n_chunks,
                n_segs=len(segments))


def _build_program(n_tab, n_blk, cap32, last_valid):
    """Emit + compile the single-core SPMD program."""
    import concourse.bacc as bacc
    import concourse.mybir as mybir
    import concourse.tile as tile
    from concourse.masks import make_identity

    f32 = mybir.dt.float32
    bf16 = mybir.dt.bfloat16
    i16 = mybir.dt.int16
    i32 = mybir.dt.int32

    lay = _layout(cap32)
    C = lay["n_chunks"]
    S = lay["n_segs"]
    segments = lay["segments"]

    nc = bacc.Bacc("TRN2", target_bir_lowering=False, debug=False,
                   num_swdge_queues=4)

    x_d = nc.dram_tensor("x", [n_tab, D], f32, kind="ExternalInput")
    idx_d = nc.dram_tensor("idx16", [P, C * 8], i16, kind="ExternalInput")
    rowoff_d = nc.dram_tensor("rowoff_t", [P, S], bf16, kind="ExternalInput")
    sw_d = nc.dram_tensor("sw_t", [P, C], f32, kind="ExternalInput")
    reprow_d = nc.dram_tensor("reprow_t", [P, C], f32, kind="ExternalInput")
    repc_d = nc.dram_tensor("repc_t", [P, C], f32, kind="ExternalInput")
    nsc_d = nc.dram_tensor("nsc_t", [P, C], f32, kind="ExternalInput")
    repsh_d = nc.dram_tensor("rep_sh", [P, n_blk], f32, kind="ExternalInput")
    degsh_d = nc.dram_tensor("deg_sh", [P, n_blk], f32, kind="ExternalInput")
    xself_d = nc.dram_tensor("x_self", [n_blk * P, D], f32, kind="ExternalInput")
    wcat_d = nc.dram_tensor("w_cat", [2 * D, D], f32, kind="ExternalInput")
    out_d = nc.dram_tensor("out", [n_blk * P, D], f32, kind="ExternalOutput")

    AL = mybir.AluOpType
    ACT = mybir.ActivationFunctionType

    # batches: within gather spans, never crossing a range boundary;
    # attach the segment id range of each batch
    batches = []  # (c0, nb, range, s0, s1)
    seg_ci = [s[0] for s in segments]
    for (r, s0s, s1s) in lay["spans"]:
        cs, ce = s0s // P, s1s // P
        for c0 in range(cs, ce, BATCH):
            nb = min(BATCH, ce - c0)
            import bisect
            sa = bisect.bisect_left(seg_ci, c0)
            sb = bisect.bisect_left(seg_ci, c0 + nb)
            batches.append((c0, nb, r, sa, sb))
    seg_max = max(sb - sa for (_, _, _, sa, sb) in batches)

    with tile.TileContext(nc) as tc:
        with (
            tc.tile_pool(name="meta", bufs=1) as meta,
            tc.tile_pool(name="idxp", bufs=4) as idxp,
            tc.tile_pool(name="gather", bufs=6) as gpool,
            tc.tile_pool(name="work", bufs=4) as wpool,
            tc.tile_pool(name="onehot", bufs=4) as opool,
            tc.tile_pool(name="const", bufs=1) as cpool,
            tc.tile_pool(name="fin", bufs=3) as fpool,
            tc.tile_pool(name="psum", bufs=4, space="PSUM") as psum,
            tc.tile_pool(name="psumT", bufs=2, space="PSUM") as psumT,
        ):
            rowoff_s = meta.tile([P, S], bf16)
            sw_s = meta.tile([P, C], f32)
            reprow_s = meta.tile([P, C], f32)
            repc_s = meta.tile([P, C], f32)
            nsc_s = meta.tile([P, C], f32)
            repsh_s = meta.tile([P, n_blk], f32)
            degsh_s = meta.tile([P, n_blk], f32)
            acc_all = meta.tile([P, n_blk, D], f32)
            wcat_s = cpool.tile([2 * D, D], f32)
            ident = cpool.tile([P, P], f32)
            iota_i = cpool.tile([P, P], i32)
            iota_f = cpool.tile([P, P], bf16)
            nc.sync.dma_start(out=rowoff_s[:], in_=rowoff_d[:])
            nc.sync.dma_start(out=sw_s[:], in_=sw_d[:])
            nc.sync.dma_start(out=reprow_s[:], in_=reprow_d[:])
            nc.sync.dma_start(out=repc_s[:], in_=repc_d[:])
            nc.sync.dma_start(out=nsc_s[:], in_=nsc_d[:])
            nc.sync.dma_start(out=repsh_s[:], in_=repsh_d[:])
            nc.sync.dma_start(out=degsh_s[:], in_=degsh_d[:])
            nc.sync.dma_start(out=wcat_s[:], in_=wcat_d[:])
            nc.vector.memset(acc_all[:].rearrange("p b d -> p (b d)"), 0.0)
            make_identity(nc, ident[:])
            nc.gpsimd.iota(iota_i[:], pattern=[[1, P]], base=0,
                           channel_multiplier=0)
            nc.vector.tensor_copy(out=iota_f[:], in_=iota_i[:])

            run_psum = {}  # block -> live psum tile for its current run

            def finalize_block(blk):
                valid = P if blk < n_blk - 1 else last_valid
                recip = fpool.tile([P, 1], f32, tag="recip")
                nc.any.tensor_scalar_add(out=recip[:],
                                         in0=degsh_s[:, blk:blk + 1],
                                         scalar1=1e-6)
                nc.vector.reciprocal(out=recip[:], in_=recip[:])
                xb = fpool.tile([P, D], f32, tag="xb")
                nc.scalar.dma_start(out=xb[:], in_=xself_d[blk * P:(blk + 1) * P, :])
                srep = fpool.tile([P, 1], f32, tag="srep")
                nc.scalar.activation(srep[:], repsh_s[:, blk:blk + 1], ACT.Sigmoid)
                cat = fpool.tile([P, 2 * D], f32, tag="cat")
                nc.scalar.mul(out=cat[:, 0:D], in_=acc_all[:, blk, :],
                              mul=recip[:])
                nc.scalar.mul(out=cat[:, D:2 * D], in_=xb[:], mul=srep[:])
                catT_ps = psumT.tile([P, P], f32, tag="catT")
                nc.tensor.transpose(out=catT_ps[:], in_=cat[:], identity=ident[:])
                catT = fpool.tile([P, P], f32, tag="catT_s")
                nc.vector.tensor_copy(out=catT[:], in_=catT_ps[:])
                out_ps = psumT.tile([P, D], f32, tag="out_ps")
                nc.tensor.matmul(out=out_ps[:], lhsT=catT[:], rhs=wcat_s[:],
                                 start=True, stop=True)
                outb = fpool.tile([P, D], f32, tag="outb")
                lk = fpool.tile([P, D], f32, tag="lk")
                nc.scalar.mul(out=lk[:], in_=out_ps[:], mul=0.01)
                nc.any.tensor_tensor(out=outb[:], in0=out_ps[:], in1=lk[:],
                                     op=AL.max)
                nc.scalar.dma_start(out=out_d[blk * P:blk * P + valid, :],
                                     in_=outb[:valid, :])

            gq = 0  # rotating SWDGE queue
            for (c0, nb, r, sa, sb) in batches:
                idx_t = idxp.tile([P, BATCH * 8], i16, tag="idx")
                nc.sync.dma_start(out=idx_t[:, :nb * 8],
                                  in_=idx_d[:, c0 * 8:(c0 + nb) * 8])
                xg = gpool.tile([P, BATCH, D], f32, tag="xg")
                for s in range(0, nb, GCH):
                    ns = min(GCH, nb - s)
                    nc.gpsimd.dma_gather(
                        out_ap=xg[:, s:s + ns, :], in_ap=x_d[r * RANGE:, :],
                        idxs_ap=idx_t[:, s * 8:(s + ns) * 8],
                        num_idxs=ns * P, num_idxs_reg=ns * P, elem_size=D,
                        queue_num=gq % 4)
                    gq += 1

                # coef = sw * sigmoid(rep_row + rep_col) * ns_col   [P, nb]
                coef = wpool.tile([P, BATCH], f32, tag="coef")
                nc.any.tensor_tensor(out=coef[:, :nb],
                                     in0=reprow_s[:, c0:c0 + nb],
                                     in1=repc_s[:, c0:c0 + nb], op=AL.add)
                nc.scalar.activation(coef[:, :nb], coef[:, :nb], ACT.Sigmoid)
                nc.any.tensor_tensor(out=coef[:, :nb], in0=coef[:, :nb],
                                     in1=sw_s[:, c0:c0 + nb], op=AL.mult)
                nc.any.tensor_tensor(out=coef[:, :nb], in0=coef[:, :nb],
                                     in1=nsc_s[:, c0:c0 + nb], op=AL.mult)

                # bf16 messages: xs = coef * x_gathered (cast fused)
                xs2 = wpool.tile([P, BATCH, D], bf16, tag="xs2")
                nc.vector.tensor_tensor(
                    out=xs2[:, :nb, :], in0=xg[:, :nb, :],
                    in1=coef[:, :nb].rearrange("p (b o) -> p b o", o=1)
                        .to_broadcast([P, nb, D]),
                    op=AL.mult)

                # 128-wide one-hots, one per segment in this batch
                nseg = sb - sa
                oh = opool.tile([P, seg_max, P], bf16, tag="oh")
                nc.vector.tensor_tensor(
                    out=oh[:, :nseg, :],
                    in0=rowoff_s[:, sa:sb]
                        .rearrange("p (b o) -> p b o", o=1)
                        .to_broadcast([P, nseg, P]),
                    in1=iota_f[:].rearrange("p (b n) -> p b n", b=1)
                        .to_broadcast([P, nseg, P]),
                    op=AL.is_equal)

                for si in range(sa, sb):
                    ci, blk, rstart, rstop, fin = segments[si]
                    if rstart:
                        run_psum[blk] = psum.tile([P, D], f32, tag="agg",
                                                  name="agg_ps")
                    nc.tensor.matmul(
                        out=run_psum[blk][:],
                        lhsT=oh[:, si - sa, :],
                        rhs=xs2[:, ci - c0, :],
                        start=rstart, stop=rstop)
                    if rstop:
                        nc.any.tensor_tensor(
                            out=acc_all[:, blk, :], in0=acc_all[:, blk, :],
                            in1=run_psum.pop(blk)[:], op=AL.add)
                    if fin:
                        finalize_block(blk)

    # blocks with no edges at all (never happens statistically)
    lay_blocks = {b for (_, b, _, _, _) in segments}
    assert len(lay_blocks) == n_blk, "empty block needs memset finalize"

    nc.compile()
    return nc


def _preprocess(x, edge_index, sim_weight, rep, node_signal):
    """Host-side layout: group edges into (core, dest block, col range) runs,
    pad to uniform 32-edge-quantum capacities, produce per-core arrays."""
    import ml_dtypes

    row = np.ascontiguousarray(edge_index[0]).astype(np.int64)
    col = np.ascontiguousarray(edge_index[1]).astype(np.int64)
    sw = np.ascontiguousarray(sim_weight).astype(np.float32)
    rep_f = np.ascontiguousarray(rep).astype(np.float32)
    ns_f = np.ascontiguousarray(node_signal).astype(np.float32)
    E = row.shape[0]
    rng_e = col // RANGE

    # Balanced row->(core, block) assignment: group rows by their per-range
    # edge-count signature and deal them round-robin over the 784 core-block
    # bins, so per-(block, range) counts are nearly equal across cores and the
    # max-over-cores run capacities carry almost no padding. Pure layout
    # (integer counting + permutation).
    n_bins = N_CORES * N_BLK
    hist = np.zeros((N_NODES, N_RANGES), dtype=np.int64)
    np.add.at(hist, (row, rng_e), 1)
    sig = ((hist[:, 0] * 64 + hist[:, 1]) * 64 + hist[:, 2]) * 64 + hist[:, 3]
    order_r = np.argsort(sig, kind="stable")
    bin_of = np.empty(N_NODES, dtype=np.int64)
    bin_of[order_r] = np.arange(N_NODES) % n_bins
    cap_bin = np.full(n_bins, P, dtype=np.int64)
    cap_bin[n_bins - N_CORES:] = LAST_VALID   # last block of each core
    cnts_b = np.bincount(bin_of, minlength=n_bins)
    spill = []
    free_b = cap_bin - cnts_b
    for b in np.where(free_b < 0)[0]:
        rows_b = np.where(bin_of == b)[0]
        spill.extend(rows_b[: (-free_b[b])])
    free_b = np.maximum(free_b, 0)
    fill_bins = np.repeat(np.arange(n_bins), free_b)
    if spill:
        bin_of[np.array(spill, dtype=np.int64)] = fill_bins[: len(spill)]
    # rank within bin -> local row
    order_b = np.argsort(bin_of, kind="stable")
    starts = np.zeros(n_bins + 1, dtype=np.int64)
    np.cumsum(np.bincount(bin_of, minlength=n_bins), out=starts[1:])
    rank_r = np.empty(N_NODES, dtype=np.int64)
    rank_r[order_b] = np.arange(N_NODES) - starts[bin_of[order_b]]
    corem = bin_of % N_CORES
    blkm = bin_of // N_CORES
    lrowm = blkm * P + rank_r
    orig_of = np.zeros((N_CORES, N_BLK * P), dtype=np.int64)
    orig_of[corem, lrowm] = np.arange(N_NODES)

    core = corem[row]
    blk = blkm[row]
    off = (lrowm[row] & 127).astype(np.float32)

    counts = np.zeros((N_CORES, N_BLK, N_RANGES), dtype=np.int64)
    np.add.at(counts, (core, blk, rng_e), 1)
    cap32 = (-(-counts.max(axis=0) // QUANT) * QUANT).astype(np.int64)

    lay = _layout(cap32)
    C = lay["n_chunks"]
    S = lay["n_segs"]
    total = lay["total_slots"]
    run_start = np.array(lay["run_start"], dtype=np.int64)  # [N_BLK, N_RANGES]

    # segment id lookup per (block, chunk)
    seg_map = np.full((N_BLK, C), -1, dtype=np.int64)
    for (b, ci), si in lay["seg_of"].items():
        seg_map[b, ci] = si

    key = (core * N_BLK + blk) * N_RANGES + rng_e
    n_groups = N_CORES * N_BLK * N_RANGES
    order = np.argsort(key, kind="stable")
    gcounts = np.bincount(key, minlength=n_groups)
    group_start = np.zeros(n_groups + 1, dtype=np.int64)
    np.cumsum(gcounts, out=group_start[1:])
    rank = np.arange(E, dtype=np.int64) - group_start[key[order]]
    ko = key[order]
    core_o = ko // (N_BLK * N_RANGES)
    blk_o = (ko // N_RANGES) % N_BLK
    rng_o = ko % N_RANGES
    lslot = run_start[blk_o, rng_o] + rank          # slot within core layout
    slot = core_o * total + lslot                   # chunk-layout position
    sid = seg_map[blk_o, lslot >> 7]
    segslot = (core_o * S + sid) * P + (lslot & 127)

    tot = N_CORES * total
    idx_flat = np.zeros(tot, dtype=np.int16)
    sw_p = np.zeros(tot, dtype=np.float32)
    reprow_p = np.zeros(tot, dtype=np.float32)
    repc_p = np.zeros(tot, dtype=np.float32)
    nsc_p = np.zeros(tot, dtype=np.float32)
    idx_flat[slot] = (col[order] - rng_o * RANGE).astype(np.int16)
    sw_p[slot] = sw[order]
    reprow_p[slot] = rep_f[row[order]]
    repc_p[slot] = rep_f[col[order]]
    nsc_p[slot] = ns_f[col[order]]

    rowoff_p = np.full(N_CORES * S * P, DUMMY_OFF, dtype=np.float32)
    rowoff_p[segslot] = off[order]

    def per_core(a):
        return np.ascontiguousarray(a.reshape(N_CORES, C, P).transpose(0, 2, 1))

    sw_t = per_core(sw_p)
    reprow_t = per_core(reprow_p)
    repc_t = per_core(repc_p)
    nsc_t = per_core(nsc_p)
    rowoff_t = np.ascontiguousarray(
        rowoff_p.reshape(N_CORES, S, P).transpose(0, 2, 1)
    ).astype(ml_dtypes.bfloat16)

    idx_w = idx_flat.reshape(N_CORES, C * 8, 16).transpose(0, 2, 1)
    idx16 = np.ascontiguousarray(np.tile(idx_w, (1, 8, 1)))

    rep_pad = np.zeros((N_CORES, N_BLK * P), dtype=np.float32)
    deg_pad = np.zeros((N_CORES, N_BLK * P), dtype=np.float32)
    deg_all = np.bincount(row, minlength=N_NODES).astype(np.float32)
    rep_pad[corem, lrowm] = rep_f
    deg_pad[corem, lrowm] = deg_all
    rep_sh = np.ascontiguousarray(
        rep_pad.reshape(N_CORES, N_BLK, P).transpose(0, 2, 1))
    deg_sh = np.ascontiguousarray(
        deg_pad.reshape(N_CORES, N_BLK, P).transpose(0, 2, 1))

    x_f = np.ascontiguousarray(x).astype(np.float32)
    x_self = np.zeros((N_CORES, N_BLK * P, D), dtype=np.float32)
    x_self[corem, lrowm] = x_f

    return (cap32, x_f, idx16, rowoff_t, sw_t, reprow_t, repc_t, nsc_t,
            rep_sh, deg_sh, x_self, orig_of)


_compiled = {}


def _get_program(cap32):
    key = (N_NODES, N_BLK, LAST_VALID, tuple(map(tuple, cap32.tolist())))
    if key not in _compiled:
        _compiled[key] = _build_program(N_NODES, N_BLK, cap32, LAST_VALID)
    return _compiled[key]


def run(x, edge_index, sim_weight, rep, node_signal, W, W_self, trace=False):
    from concourse.bass_utils import run_bass_kernel_spmd

    (cap32, x_f, idx16, rowoff_t, sw_t, reprow_t, repc_t, nsc_t, rep_sh,
     deg_sh, x_self, orig_of) = _preprocess(x, edge_index, sim_weight, rep,
                                            node_signal)
    w_cat = np.ascontiguousarray(
        np.concatenate([np.asarray(W, dtype=np.float32),
                        np.asarray(W_self, dtype=np.float32)], axis=0))
    nc = _get_program(cap32)
    in_maps = []
    for c in range(N_CORES):
        in_maps.append({
            "x": x_f,
            "idx16": idx16[c],
            "rowoff_t": rowoff_t[c],
            "sw_t": sw_t[c],
            "reprow_t": reprow_t[c],
            "repc_t": repc_t[c],
            "nsc_t": nsc_t[c],
            "rep_sh": rep_sh[c],
            "deg_sh": deg_sh[c],
            "x_self": x_self[c],
            "w_cat": w_cat,
        })
    res = run_bass_kernel_spmd(nc, in_maps, core_ids=list(range(N_CORES)),
                               trace=trace)
    out = np.empty((N_NODES, D), dtype=np.float32)
    for c in range(N_CORES):
        out[orig_of[c, :N_LOC]] = res.results[c]["out"][:N_LOC]
    return out, res


def kernel(x, edge_index, sim_weight, rep, node_signal, W, W_self):
    out, _ = run(x, edge_index, sim_weight, rep, node_signal, W, W_self)
    return out
